# revision 36
# baseline (speedup 1.0000x reference)
"""CRD loss kernel for 8 Trainium2 NeuronCores.

Math notes (derived from the CRDLoss reference):
  - neg_scores gathers student rows idx[i,j] = j + (j>=i) which only ever
    touches student rows 0..10 ("head"); the rest of the student projection
    (and all logits / contrast_idx / idx inputs) are dead.
  - scores[i, :] for i>=11 is just anchor[i] @ s_head[0:10].T, a matmul.
    Rows 0..10 (on the shard owning them) need a shifted-head correction.
  - sum(log_D1)+sum(log_D0) = sum_i s_pos_i/T + 9*N*log(m*Pn)
                              - sum_{i,j} log(exp(s_ij/T) + m*Pn + EPS)
    so each core only returns per-(block,j) log-sums and pos-score sums; the
    host combines the 8 cores' partials into the two scalar losses.

Device layout per core (rows sharded 2048/core):
  - anchor features host-transposed to [4(blk), 128(k_in), 8(kt), 512(r)]
    fp8, block-major so each 512-row block is one contiguous 512KB DMA that
    overlaps with compute on the previous block (sync HWDGE ring).  All
    small constants are packed into 5 tensors DMA'd on the Act HWDGE ring
    so the x stream starts immediately (HWDGE dispatch is ~0.6us per
    dma_start regardless of size, so 19 small DMAs would stall the queue).
  - projections use fp8 DoubleRow matmuls (2 k-tiles per instruction) which
    roughly halves PE streaming time vs one matmul per k-tile.
  - the four row-blocks' score/norm matmuls are column-tiled (tile_position
    (0,32b)) into one [128,512] PSUM tile so the whole tail (rsqrt, scale,
    exp, log, reduce) runs as a few full-width ops per tensor.
  - 1/sqrt is computed as Exp(-0.5*Ln(x)); a manually planted
    InstLoadActFuncSet(natural_log_exp_and_others) makes ALL activations
    (Ln/Exp) resolve against one table so only a single ACT_TABLE_LOAD is
    ever issued (the automatic chooser alternates exp/ln-only tables, which
    cost ~11 reloads x 2.7us in earlier versions).
"""

import sys

for _p in ("/opt/trn_rl_repo", "/root/.axon_site/_ro/trn_rl_repo"):
    if _p not in sys.path:
        sys.path.insert(0, _p)

import math

import ml_dtypes
import numpy as np

import concourse.bass as bass  # noqa: F401
import concourse.tile as tile
from concourse import bacc, mybir
from concourse.bass_utils import run_bass_kernel_spmd

F32 = mybir.dt.float32
F32R = mybir.dt.float32r
BF16 = mybir.dt.bfloat16
FP8 = mybir.dt.float8e4
WSCALE = 64.0
AF = mybir.ActivationFunctionType
DR = mybir.MatmulPerfMode.DoubleRow

EPS = 1e-07
K = 10
T = 0.07
DIN = 1024
DOUT = 128
N = 16384
NCORES = 8
SH = N // NCORES          # 2048 rows per core
NKT = DIN // 128          # 8 k-tiles
BLK = 512
NBLK = SH // BLK          # 4 row blocks per core
NH = 16                   # head rows shipped (11 used)

def _ln_exp_set_id(arch):
    # Find the act-table set serving BOTH Exp and Ln (natural_log_exp_and
    # _others, id 6 in the current neuronxcc) so one load covers every
    # activation below.  Resolved dynamically for robustness; the
    # insert_act_table_loads pass checks the same table contents, so a
    # mismatch would only add loads, never break correctness.
    try:
        from concourse.hw_specs import get_activation_tables
        for i, fns in enumerate(get_activation_tables(arch).values()):
            if AF.Exp in fns and AF.Ln in fns:
                return i
    except Exception:
        pass
    return 6

# (anchor feature, anchor W, anchor b, side) per combo; side E=0 uses the
# entity student head, side R=1 the rel student head.
COMBOS = [
    ("entity_features_TeaE", "We_tE", "be_tE", 0),
    ("entity_features_TeaR", "We_tR", "be_tR", 0),
    ("rel_features_TeaE", "Wr_tE", "br_tE", 1),
    ("rel_features_TeaR", "Wr_tR", "br_tR", 1),
]
HEADS = [("entity_features_s", "We_s", "be_s"), ("rel_features_s", "Wr_s", "br_s")]

_CACHE = {}


def _build(c_const):
    """Build + compile the SPMD program. c_const = m*Pn + EPS baked into Ln."""
    nc = bacc.Bacc("TRN2", target_bir_lowering=False, debug=False)

    xdr = [nc.dram_tensor(f"x{q}", [NBLK, 128, NKT, BLK], FP8,
                          kind="ExternalInput") for q in range(4)]
    wpkdr = nc.dram_tensor("wpk", [128, NKT, 4 * DOUT], FP8,
                           kind="ExternalInput")
    whpkdr = nc.dram_tensor("whpk", [128, NKT, 2 * DOUT], BF16,
                            kind="ExternalInput")
    hpkdr = nc.dram_tensor("hpk", [128, NKT, 2 * NH], BF16,
                           kind="ExternalInput")
    bpkdr = nc.dram_tensor("bpk", [DOUT, 6], F32, kind="ExternalInput")
    mfdr = nc.dram_tensor("mf", [NH, NH + 1], F32, kind="ExternalInput")
    outdr = nc.dram_tensor("out", [128, 8], F32, kind="ExternalOutput")

    ln_invT = float(math.log(1.0 / T))

    with tile.TileContext(nc) as tc:
        # Plant the combined exp+ln table as the scalar queue's first
        # instruction; insert_act_table_loads' fixpoint then sees every
        # activation already served and inserts nothing else.
        nc.scalar.add_instruction(
            mybir.InstLoadActFuncSet(
                name=nc.get_next_instruction_name(),
                act_func_set_id=_ln_exp_set_id(nc.m.arch), ins=[], outs=[],
            )
        )
        with (
            tc.tile_pool(name="consts", bufs=1) as consts,
            tc.tile_pool(name="xp", bufs=16) as xp,
            tc.tile_pool(name="mid", bufs=6) as mid,
            tc.tile_pool(name="sco", bufs=2) as scop,
            tc.tile_pool(name="tiny", bufs=4) as tinyp,
            tc.tile_pool(name="pacc", bufs=3, space="PSUM") as pacc,
            tc.tile_pool(name="psco", bufs=2, space="PSUM") as psco,
            tc.tile_pool(name="pnsq", bufs=2, space="PSUM") as pnsq,
            tc.tile_pool(name="ptiny", bufs=1, space="PSUM") as ptiny,
        ):
            # ---- packed constants (5 DMAs on the Act HWDGE ring) ----
            wpk_t = consts.tile([128, NKT, 4 * DOUT], FP8, tag="wpk")
            whpk_t = consts.tile([128, NKT, 2 * DOUT], BF16, tag="whpk")
            hpk_t = consts.tile([128, NKT, 2 * NH], BF16, tag="hpk")
            bpk_t = consts.tile([DOUT, 6], F32, tag="bpk")
            mf_t = consts.tile([NH, NH + 1], F32, tag="mf")
            # wpk rides first on the sync ring (ahead of the x stream) so the
            # first projection matmul is never gated on the Act ring's queue.
            nc.sync.dma_start(out=wpk_t[:], in_=wpkdr[:])
            nc.scalar.dma_start(out=whpk_t[:], in_=whpkdr[:])
            nc.scalar.dma_start(out=hpk_t[:], in_=hpkdr[:])
            nc.scalar.dma_start(out=bpk_t[:], in_=bpkdr[:])
            nc.scalar.dma_start(out=mf_t[:], in_=mfdr[:])

            def w_ap(q, ktlo, kthi):
                return wpk_t[:, ktlo:kthi, DOUT * q:DOUT * (q + 1)]

            def wh_ap(s, kt):
                return whpk_t[:, kt, DOUT * s:DOUT * (s + 1)]

            def h_ap(s, kt):
                return hpk_t[:, kt, NH * s:NH * (s + 1)]

            def b_ap(q):
                return bpk_t[:, 2 + q:3 + q]

            def bh_ap(s):
                return bpk_t[:, s:s + 1]

            mu_ap = mf_t[0:10, 0:11]
            fl_ap = mf_t[0:10, NH:NH + 1]

            # on-device generated constants
            ones_kn = consts.tile([128, 32], BF16, tag="ones_kn")
            ones_knr = consts.tile([128, NH], F32, tag="ones_knr")
            ones_1p = consts.tile([1, 128], F32, tag="ones_1p")
            outacc = consts.tile([128, 8], F32, tag="outacc")
            cb_t = consts.tile([128, 1], F32, tag="cb")
            lt_t = consts.tile([128, 1], F32, tag="lt")
            shead = [consts.tile([128, 32], BF16, name=f"shead{s}", tag=f"shead{s}")
                     for s in range(2)]
            sheadsh = [consts.tile([128, NH], BF16, name=f"sheadsh{s}",
                                   tag=f"sheadsh{s}") for s in range(2)]
            nc.vector.memset(ones_kn[:], 1.0)
            nc.vector.memset(ones_knr[:], 1.0)
            nc.vector.memset(ones_1p[:], 1.0)
            nc.vector.memset(outacc[:], 0.0)
            nc.vector.memset(cb_t[:], float(c_const))
            nc.vector.memset(lt_t[:], ln_invT)
            acc_col = [outacc[:, q:q + 1] for q in range(4)]
            pos_col = [outacc[:, 4 + q:5 + q] for q in range(4)]

            # ---- PE warm-up: ~10 junk matmuls on a memset tile so the HAM
            # clock gate releases (1.2->2.4 GHz) before the real stream ----
            jr = consts.tile([128, BLK], BF16, tag="junk")
            nc.vector.memset(jr[:], 0.5)
            for _ in range(20):
                j_ps = pacc.tile([128, BLK], F32, tag="pacc")
                nc.tensor.matmul(out=j_ps[:], lhsT=jr[:, 0:128], rhs=jr[:],
                                 start=True, stop=True)

            # ---- student heads: normalized s_head^T [128(dout), 32].
            # Emitted between anchor 0's projections and its sco matmuls so
            # the in-order PE/Vector queues never park on the whpk/hpk DMAs
            # while the x stream is already resident (head-of-line block).
            def head_phase():
                yhs, lnhs, inv1s = [], [], []
                for s in range(2):
                    yh_ps = ptiny.tile([128, 32], F32, name="yh_ps", tag="ptiny")
                    for kt in range(NKT):
                        nc.tensor.matmul(
                            out=yh_ps[:, 0:NH],
                            lhsT=wh_ap(s, kt),
                            rhs=h_ap(s, kt),
                            start=(kt == 0),
                            stop=(kt == NKT - 1),
                        )
                    yh = tinyp.tile([128, NH], F32, name=f"yh{s}", tag=f"yh{s}")
                    nc.vector.tensor_scalar_add(out=yh[:], in0=yh_ps[:, 0:NH],
                                                scalar1=bh_ap(s))
                    sqh = tinyp.tile([128, NH], F32R, name="sqh", tag=f"sqh{s}")
                    nc.vector.tensor_mul(out=sqh[:], in0=yh[:], in1=yh[:])
                    nsqh_ps = ptiny.tile([128, 32], F32, name="nsqh_ps",
                                         tag="ptiny")
                    nc.tensor.matmul(
                        out=nsqh_ps[0:NH, 0:NH],
                        lhsT=ones_knr[:].bitcast(F32R),
                        rhs=sqh[:],
                        start=True,
                        stop=True,
                    )
                    # 1/sqrt(v) = Exp(-0.5*Ln(v)) — ScalarE stays on Exp/Ln
                    lnh = tinyp.tile([1, NH], F32, name=f"lnh{s}", tag=f"lnh{s}")
                    nc.scalar.activation(out=lnh[:], in_=nsqh_ps[0:1, 0:NH],
                                         func=AF.Ln)
                    yhs.append(yh)
                    lnhs.append(lnh)
                for s in range(2):
                    inv1 = tinyp.tile([1, NH], F32R, name=f"inv1{s}",
                                      tag=f"inv1{s}")
                    nc.scalar.activation(out=inv1[:], in_=lnhs[s][:],
                                         func=AF.Exp, scale=-0.5)
                    inv1s.append(inv1)
                for s in range(2):
                    invb_ps = ptiny.tile([128, 32], F32, name="invb_ps",
                                         tag="ptiny")
                    nc.tensor.matmul(
                        out=invb_ps[0:128, 0:NH],
                        lhsT=ones_1p[:].bitcast(F32R),
                        rhs=inv1s[s][:],
                        start=True,
                        stop=True,
                    )
                    nc.vector.tensor_mul(out=shead[s][:, 0:NH], in0=yhs[s][:],
                                         in1=invb_ps[:, 0:NH])
                    # pad cols 16..31 (fills the 32-wide col-tile strip)
                    nc.vector.tensor_copy(out=shead[s][:, NH:32],
                                          in_=shead[s][:, 0:NH])
                    # shifted head (col j = head j+1) for the local correction
                    nc.vector.tensor_copy(out=sheadsh[s][:, 0:NH - 1],
                                          in_=shead[s][:, 1:NH])
                    nc.vector.tensor_copy(out=sheadsh[s][:, NH - 1:NH],
                                          in_=shead[s][:, 0:1])

            # ---- main: 4 anchors, 4 col-tiled row-blocks each.  Each
            # anchor's tail is EMITTED two blocks into the next anchor so the
            # in-order Vector/Scalar queues never head-of-line block the next
            # anchor's yb/sq behind a tail op that waits on ScalarE. ----
            def make_split_tail(q, sco_ps, nsq_ps, s1_ps):
                # Column-halved, software-pipelined tail for the LAST anchor:
                # its chain is fully exposed after the final matmul, and two
                # 256-col halves overlap Ln/Exp stages across ScalarE/DVE.
                def tail():
                    H = BLK // 2
                    rln = scop.tile([128, BLK], F32, tag="rln")
                    rsq = scop.tile([128, BLK], F32, tag="rsq")
                    st = scop.tile([128, BLK], F32, tag="st")
                    ex = scop.tile([128, BLK], F32, tag="ex")
                    laccs = []
                    for h in range(2):
                        c = slice(H * h, H * h + H)
                        nc.scalar.activation(out=rln[:, c], in_=nsq_ps[:, c],
                                             func=AF.Ln)
                        nc.scalar.activation(out=rsq[:, c], in_=rln[:, c],
                                             func=AF.Exp, scale=-0.5,
                                             bias=lt_t[:])
                        nc.vector.tensor_mul(out=st[:, c], in0=sco_ps[:, c],
                                             in1=rsq[:, c])
                        if h == 0:
                            d = tinyp.tile([NH, NH], F32, tag="d")
                            nc.vector.tensor_mul(out=d[0:10, 0:11],
                                                 in0=s1_ps[0:10, 0:11],
                                                 in1=rsq[0:10, 0:11])
                            nc.vector.tensor_sub(out=d[0:10, 0:11],
                                                 in0=d[0:10, 0:11],
                                                 in1=st[0:10, 0:11])
                            nc.vector.tensor_mul(out=d[0:10, 0:11],
                                                 in0=d[0:10, 0:11],
                                                 in1=mu_ap)
                            nc.vector.tensor_add(out=st[0:10, 0:11],
                                                 in0=st[0:10, 0:11],
                                                 in1=d[0:10, 0:11])
                        nc.scalar.activation(out=ex[:, c], in_=st[:, c],
                                             func=AF.Exp)
                        lacc = tinyp.tile([128, 1], F32, tag=f"lacc{h}")
                        nc.scalar.activation(out=ex[:, c], in_=ex[:, c],
                                             func=AF.Ln, bias=cb_t[:],
                                             accum_out=lacc[:])
                        laccs.append(lacc)
                    posr = tinyp.tile([128, 1], F32, tag="posr")
                    nc.vector.reduce_sum(out=posr[:], in_=st[:],
                                         axis=mybir.AxisListType.X)
                    nc.vector.tensor_add(out=acc_col[q], in0=acc_col[q],
                                         in1=laccs[0][:])
                    nc.vector.tensor_add(out=acc_col[q], in0=acc_col[q],
                                         in1=laccs[1][:])
                    nc.vector.tensor_add(out=pos_col[q], in0=pos_col[q],
                                         in1=posr[:])
                return tail

            def make_tail(q, sco_ps, nsq_ps, s1_ps, split=False):
                if split:
                    return make_split_tail(q, sco_ps, nsq_ps, s1_ps)
                def tail():
                    # full-width ops covering all 4 col-tiled blocks at once
                    rln = scop.tile([128, BLK], F32, tag="rln")
                    nc.scalar.activation(out=rln[:], in_=nsq_ps[:], func=AF.Ln)
                    rsq = scop.tile([128, BLK], F32, tag="rsq")
                    nc.scalar.activation(out=rsq[:], in_=rln[:], func=AF.Exp,
                                         scale=-0.5, bias=lt_t[:])
                    st = scop.tile([128, BLK], F32, tag="st")
                    nc.vector.tensor_mul(out=st[:], in0=sco_ps[:], in1=rsq[:])

                    # local rows 0..10 shifted-head correction (mu_ap is
                    # already mu*flag from the host, zero on cores != 0)
                    d = tinyp.tile([NH, NH], F32, tag="d")
                    nc.vector.tensor_mul(out=d[0:10, 0:11],
                                         in0=s1_ps[0:10, 0:11],
                                         in1=rsq[0:10, 0:11])
                    nc.vector.tensor_sub(out=d[0:10, 0:11], in0=d[0:10, 0:11],
                                         in1=st[0:10, 0:11])
                    nc.vector.tensor_mul(out=d[0:10, 0:11], in0=d[0:10, 0:11],
                                         in1=mu_ap)
                    nc.vector.tensor_add(out=st[0:10, 0:11],
                                         in0=st[0:10, 0:11],
                                         in1=d[0:10, 0:11])

                    ex = scop.tile([128, BLK], F32, tag="ex")
                    nc.scalar.activation(out=ex[:], in_=st[:], func=AF.Exp)
                    lacc = tinyp.tile([128, 1], F32, tag="lacc")
                    nc.scalar.activation(out=ex[:], in_=ex[:], func=AF.Ln,
                                         bias=cb_t[:], accum_out=lacc[:])
                    posr = tinyp.tile([128, 1], F32, tag="posr")
                    nc.vector.reduce_sum(out=posr[:], in_=st[:],
                                         axis=mybir.AxisListType.X)
                    nc.vector.tensor_add(out=acc_col[q], in0=acc_col[q],
                                         in1=lacc[:])
                    nc.vector.tensor_add(out=pos_col[q], in0=pos_col[q],
                                         in1=posr[:])
                return tail

            def emit_block(q, blk, sco_ps, nsq_ps, do_sco=True):
                """proj + yb + sq + nsq (+sco) for one 512-row block."""
                xb = xp.tile([128, NKT, BLK], FP8, name=f"x{q}b{blk}", tag="x")
                nc.sync.dma_start(out=xb[:], in_=xdr[q][blk])
                acc_ps = pacc.tile([128, BLK], F32, tag="pacc")
                for ktp in range(NKT // 2):
                    nc.tensor.matmul(
                        out=acc_ps[:],
                        lhsT=w_ap(q, 2 * ktp, 2 * ktp + 2),
                        rhs=xb[:, 2 * ktp:2 * ktp + 2, :],
                        start=(ktp == 0),
                        stop=(ktp == NKT // 2 - 1),
                        perf_mode=DR,
                    )
                yb = mid.tile([128, BLK], BF16, tag="yb")
                nc.vector.tensor_scalar_add(out=yb[:], in0=acc_ps[:],
                                            scalar1=b_ap(q))
                sq = mid.tile([128, BLK], BF16, tag="sq")
                # On DVE right after yb: the Scalar FIFO would interleave
                # ~700ns tail activations ahead of the square and delay the
                # nsq matmul; the deferred tails keep the DVE queue clear.
                nc.vector.tensor_mul(out=sq[:], in0=yb[:], in1=yb[:])
                # sco BEFORE nsq: sco only needs yb while nsq needs the
                # longer yb->sq chain; the in-order PE queue would otherwise
                # park on nsq and delay both.
                if do_sco:
                    emit_sco(q, blk, sco_ps, yb)
                nc.tensor.matmul(
                    out=nsq_ps[32 * blk:32 * blk + 32, :],
                    lhsT=ones_kn[:],
                    rhs=sq[:],
                    start=True,
                    stop=True,
                    tile_position=(0, 32 * blk),
                )
                return yb

            def emit_sco(q, blk, sco_ps, yb, s1_ps=None):
                s = COMBOS[q][3]
                nc.tensor.matmul(
                    out=sco_ps[32 * blk:32 * blk + 32, :],
                    lhsT=shead[s][:],
                    rhs=yb[:],
                    start=True,
                    stop=True,
                    tile_position=(0, 32 * blk),
                )
                if s1_ps is not None:
                    # shifted-head scores for the local-rows correction
                    nc.tensor.matmul(
                        out=s1_ps[0:NH, 0:NH],
                        lhsT=sheadsh[s][:],
                        rhs=yb[:, 0:NH],
                        start=True,
                        stop=True,
                    )

            # ---- anchor 0: projections first (they only need wpk + x which
            # land before the head DMAs), the head phase while they run, then
            # anchor 0's sco matmuls from the kept yb tiles ----
            sco_ps = psco.tile([128, BLK], F32, tag="psco")
            nsq_ps = pnsq.tile([128, BLK], F32, tag="pnsq")
            s1_ps = ptiny.tile([128, 32], F32, name="s1_ps", tag="ptiny")
            q0_yb = [emit_block(0, blk, sco_ps, nsq_ps, do_sco=False)
                     for blk in range(NBLK)]
            head_phase()
            emit_sco(0, 0, sco_ps, q0_yb[0], s1_ps=s1_ps)
            for blk in range(1, NBLK):
                emit_sco(0, blk, sco_ps, q0_yb[blk])
            pending_tail = make_tail(0, sco_ps, nsq_ps, s1_ps)

            for q in range(1, 4):
                sco_ps = psco.tile([128, BLK], F32, tag="psco")
                nsq_ps = pnsq.tile([128, BLK], F32, tag="pnsq")
                s1_ps = ptiny.tile([128, 32], F32, name="s1_ps", tag="ptiny")
                for blk in range(NBLK):
                    yb = emit_block(q, blk, sco_ps, nsq_ps,
                                    do_sco=(blk != 0))
                    if blk == 0:
                        emit_sco(q, 0, sco_ps, yb, s1_ps=s1_ps)
                    if blk == 2 and pending_tail is not None:
                        pending_tail()
                        pending_tail = None
                pending_tail = make_tail(q, sco_ps, nsq_ps, s1_ps,
                                         split=(q == 3))
            pending_tail()

            nc.sync.dma_start(out=outdr[:], in_=outacc[:])

    nc.compile()

    n_loads = sum(
        isinstance(inst, mybir.InstLoadActFuncSet)
        for blk in nc.main_func.blocks for inst in blk.instructions
    )
    if n_loads != 1:
        print(f"WARNING: expected 1 act table load, got {n_loads}")
    return nc


def _pack_x(feat):
    """[B,TS,DIN] f32 -> per-core [NBLK, 128, NKT, BLK] fp8, block-major."""
    f = np.ascontiguousarray(np.asarray(feat, dtype=np.float32)).reshape(N, DIN)
    # (core, blk, r, kt, p) -> (core, blk, p, kt, r)
    v = f.reshape(NCORES, NBLK, BLK, NKT, 128).transpose(0, 1, 4, 3, 2)
    return np.ascontiguousarray(v.astype(ml_dtypes.float8_e4m3))


def _pack_w(w):
    v = np.asarray(w, dtype=np.float32).reshape(NKT, 128, DOUT).transpose(1, 0, 2)
    return np.ascontiguousarray(v.astype(ml_dtypes.bfloat16))


def _pack_w8(w):
    # x64 lands typical N(0, 0.02^2) weights in the fp8 normal range; the scale
    # cancels in the L2 normalization (biases scaled to match).
    v = (np.asarray(w, dtype=np.float32) * WSCALE).reshape(NKT, 128, DOUT)
    return np.ascontiguousarray(v.transpose(1, 0, 2).astype(ml_dtypes.float8_e4m3))


def _pack_h(feat):
    f = np.asarray(feat, dtype=np.float32).reshape(N, DIN)[0:NH]  # [16, 1024]
    v = f.T.reshape(NKT, 128, NH).transpose(1, 0, 2)
    return np.ascontiguousarray(v.astype(ml_dtypes.bfloat16))


def kernel(**inputs):
    M = int(np.asarray(inputs["M"]))
    m = K - 1
    Pn = 1.0 / float(M)
    c_const = m * Pn + EPS

    key = ("v19", M)
    if key not in _CACHE:
        _CACHE[key] = _build(c_const)
    nc = _CACHE[key]

    xs = [_pack_x(inputs[COMBOS[q][0]]) for q in range(4)]
    wpk = np.concatenate([_pack_w8(inputs[COMBOS[q][1]]) for q in range(4)],
                         axis=2)
    whpk = np.concatenate([_pack_w(inputs[HEADS[s][1]]) for s in range(2)],
                          axis=2)
    hpk = np.concatenate([_pack_h(inputs[HEADS[s][0]]) for s in range(2)],
                         axis=2)
    bcols = [np.asarray(inputs[HEADS[s][2]], dtype=np.float32).reshape(DOUT, 1)
             for s in range(2)]
    bcols += [np.asarray(inputs[COMBOS[q][2]], dtype=np.float32).reshape(DOUT, 1)
              * WSCALE for q in range(4)]
    bpk = np.ascontiguousarray(np.concatenate(bcols, axis=1))

    j = np.arange(NH)[:, None]
    i = np.arange(NH)[None, :]
    mu = (j >= i).astype(np.float32)  # 1 where the shifted head row is used

    in_maps = []
    for cid in range(NCORES):
        # fold the "only core 0 owns rows 0..10" flag into the mask
        mf = np.concatenate(
            [mu * (1.0 if cid == 0 else 0.0), np.zeros((NH, 1), np.float32)],
            axis=1)
        im = {"wpk": wpk, "whpk": whpk, "hpk": hpk, "bpk": bpk,
              "mf": np.ascontiguousarray(mf)}
        for q in range(4):
            im[f"x{q}"] = xs[q][cid]
        in_maps.append(im)

    res = run_bass_kernel_spmd(nc, in_maps, list(range(NCORES)))
    global LAST_RESULT
    LAST_RESULT = res

    outs = np.stack([np.asarray(res.results[cid]["out"])
                     for cid in range(NCORES)])  # [8, 128, 8]
    rows_log = np.concatenate([32 * b + np.arange(10) for b in range(NBLK)])
    rows_pos = np.array([32 * b for b in range(NBLK)])
    slog = outs[:, rows_log, 0:4].sum(axis=(0, 1))    # [4]
    spos_T = outs[:, rows_pos, 4:8].sum(axis=(0, 1))  # [4], already / T
    const = 9.0 * N * np.log(m * Pn)
    loss = -(spos_T + const - slog) / N                # [4]
    return np.array([loss[0] + loss[1], loss[2] + loss[3]], dtype=np.float32)


if __name__ == "__main__":
    rng = np.random.default_rng(0)
    fake = {}
    for nm in ("entity_features_s", "rel_features_s", "entity_features_TeaE",
               "rel_features_TeaE", "entity_features_TeaR", "rel_features_TeaR"):
        fake[nm] = rng.standard_normal((16, 1024, DIN), dtype=np.float32)
    for nm in ("entity_logits_TeaE", "rel_logits_TeaE", "entity_logits_TeaR",
               "rel_logits_TeaR"):
        fake[nm] = rng.standard_normal((16, 1024, 100), dtype=np.float32)
    for pn in ("We_s", "We_tE", "We_tR", "Wr_s", "Wr_tE", "Wr_tR"):
        fake[pn] = (rng.standard_normal((DIN, DOUT), dtype=np.float32) * 0.02)
        fake[pn.replace("W", "b", 1)] = np.zeros((DOUT,), np.float32)
    fake["contrast_idx"] = rng.integers(0, 50000, size=(N,))
    fake["idx"] = rng.integers(0, 50000, size=(N,))
    fake["M"] = 50000
    print(kernel(**fake))
